# revision 23
# baseline (speedup 1.0000x reference)
"""Trainium2 Bass kernel for the vq_codebook problem.

  dist_sq[n,k] = sum_d (x[n,d]-ctrs[k,d])^2 * s[d]
  out = softmax(-dist_sq, axis=1) @ values

Sharding: data-parallel over N (8192 rows of x per core); ctrs/values/s
replicated on all 8 cores. No collectives (forward only).

Math trick: softmax is shift-invariant, so
  softmax(-dist_sq)[n,k] = softmax(2*cross_s[n,k] - c_sq[k])  with
  cross_s = (x*s) @ ctrs.T,  c_sq[k] = sum_d s[d]*ctrs[k,d]^2.
We compute E = exp(2*cross_s - c_sq) unnormalized (range-checked: max
exponent ~48 < 88, row-max min ~ -27, so fp32 exp never overflows and
denominators stay normal), then
  y[n,:] = (E.T @ values_aug)[n,:256] / (E.T @ values_aug)[n,256]
with values_aug = [values | ones] so the denominator comes from the same
accumulating matmul.

Layouts: phase 1 runs transposed (k on partitions, n on free): per
128-centroid chunk, stationary lhs1 = fp16(s*ctrs^T) [64, 128] against
the moving fp16 x^T [64, 512]. The -c_sq shift is applied as the exp
activation's per-partition f32 bias (partitions = k in this layout), so
it costs nothing on the PE and carries no fp16 quantization error. To
share one bias per activation, each exp covers the same chunk of TWO
row-tiles ([128, 2, 512]); the two matmuls of such a pair also share
the same stationary, halving weight loads. Phase-1 fp16 operands add
~4e-3 rel err vs the 2e-2 budget (validated numerically); fp16 gets PE
fast-weight-load so LDWEIGHTS hides under the previous matmul stream.

x^T and ctrs^T are produced by the DMA XBAR transpose engine (2-byte
dtypes), keeping the PE entirely free of transpose work. The XBAR
maps in[128, (blk 128)] -> out[d, blk, p] = in[p, blk*128 + d] (middle
output dim strides the source free index by 128 = the XBAR tile width
— verified against hardware), so x is staged in a padded [*, 128]
free layout with data in cols 0..63; the junk columns transpose into
partitions 64..127, which no access pattern ever reads.

Phase 2 uses bf16 E chunks as the stationary operand against
values_aug, producing y in natural [n, d_out] layout (fp8/DoubleRow was
evaluated numerically and busts the error budget; bf16 is the floor).

Pipelining: engine queues are in-order, so phase-1 chunk-pairs and
phase-2 sub-tiles are interleaved 1:1 in emission order — while the ACT
engine drains an exp (~1us), the PE streams the previous pair's phase-2
matmuls instead of stalling on the psA pool. The lhs1/c_sq prep (DVE +
ACT only) is likewise interleaved with pair-0 chunk by chunk.
"""

import os

os.environ.setdefault("JAX_PLATFORMS", "axon")

import numpy as np

N, D_IN, K, D_OUT = 65536, 64, 1024, 256
NCORES = 8
NS = N // NCORES  # 8192 rows per core
TROWS = 512  # rows of x per tile
NTILES = NS // TROWS  # 16
NPAIRS = NTILES // 2  # tiles are processed in pairs sharing exp bias
KC = K // 128  # 8 centroid chunks
NSUB = TROWS // 128  # 4 output sub-tiles per tile

USE_F32R = True

_cache = {}


def _build(rows=NS):
    import concourse.bacc as bacc
    import concourse.tile as tile
    from concourse import mybir

    f32 = mybir.dt.float32
    f16 = mybir.dt.float16
    bf16 = mybir.dt.bfloat16
    Exp = mybir.ActivationFunctionType.Exp
    Copy = mybir.ActivationFunctionType.Copy
    Mult = mybir.AluOpType.mult
    Add = mybir.AluOpType.add

    ntiles = rows // TROWS
    npairs = ntiles // 2
    nc = bacc.Bacc("TRN2", target_bir_lowering=False, debug=False)
    x = nc.declare_dram_parameter("x", [rows, D_IN], f32, isOutput=False)
    ctrs = nc.declare_dram_parameter("ctrs", [K, D_IN], f32, isOutput=False)
    values = nc.declare_dram_parameter("values", [K, D_OUT], f32, isOutput=False)
    s = nc.declare_dram_parameter("s", [D_IN], f32, isOutput=False)
    y = nc.declare_dram_parameter("y", [rows, D_OUT], f32, isOutput=True)

    with tile.TileContext(nc) as tc:
        with (
            tc.tile_pool(name="const", bufs=1) as constp,
            tc.tile_pool(name="tmp1", bufs=2) as tmp1p,
            tc.tile_pool(name="xt32", bufs=4) as xt32p,
            tc.tile_pool(name="xsT", bufs=4) as xsTp,
            tc.tile_pool(name="E", bufs=2) as Ep,
            tc.tile_pool(name="ysb", bufs=3) as yp,
            tc.tile_pool(name="rcp", bufs=8) as rcpp,
            tc.tile_pool(name="psA", bufs=2, space="PSUM") as psA,
            tc.tile_pool(name="psO", bufs=3, space="PSUM") as psO,
        ):
            # ---- startup: all plain DMAs on the gpsimd swdge ring; the sync
            # hwdge ring carries only the XBAR transposes, which gate each
            # pair's phase 1. Startup critical path avoids the PE entirely
            # (engine preambles take ~6us; the PE must not be a producer).
            def issue_x_dma(i):
                n0 = i * TROWS
                xt32 = xt32p.tile([128, NSUB, D_IN], f32)
                nc.gpsimd.dma_start(
                    xt32[:], x[n0 : n0 + TROWS, :].rearrange("(a p) d -> p a d", p=128)
                )
                return xt32

            ctrs32 = constp.tile([128, KC, D_IN], f32)
            nc.gpsimd.dma_start(
                ctrs32[:], ctrs[:].rearrange("(c p) d -> p c d", p=128)
            )
            xt_inflight = [issue_x_dma(0), issue_x_dma(1)]
            s_row = constp.tile([1, D_IN], f32)
            nc.gpsimd.dma_start(s_row[:], s[:].rearrange("(o d) -> o d", o=1))

            # ---------- per-tile x pipeline: cast + XBAR transpose ----------
            # Persistent double-buffered fp16 staging so the pad columns
            # (64..127, junk after transpose) are zeroed exactly once.
            xt16_all = constp.tile([128, 2, NSUB, 128], f16)
            nc.vector.memset(xt16_all[:, :, :, D_IN:128], 0.0)

            def assemble_xsT(xt32, i):
                slot = i % 2
                nc.vector.tensor_copy(xt16_all[:, slot, :, 0:D_IN], xt32[:])
                xsT = xsTp.tile([128, NSUB, 128], f16)
                # out[d, a, p] = in[p, a*128 + d]  (hardware XBAR mapping);
                # x lives in cols 0..63 of each 128 block -> rows 0..63.
                nc.sync.dma_start_transpose(
                    xsT[:], xt16_all[:, slot, :, :].rearrange("p a d -> p (a d)")
                )
                return xsT

            # pair-0 x path first: its XBARs lead the sync ring
            xsT_cur = [
                assemble_xsT(xt_inflight[0], 0),
                assemble_xsT(xt_inflight[1], 1),
            ]

            # s broadcast along partitions on the gpsimd engine (no PE in
            # the startup chain), then (s*ctrs) in fp16 padded layout; the
            # XBAR transpose of that IS the phase-1 stationary lhs1, and
            # c_sq below reuses the same quantized product.
            sbc = constp.tile([128, D_IN], f32)
            nc.gpsimd.partition_broadcast(sbc[:], s_row[:])
            sc16 = constp.tile([128, KC, 128], f16)
            nc.gpsimd.memset(sc16[:, :, D_IN:128], 0.0)
            negcsq = constp.tile([128, KC], f32)
            for c in range(KC):
                nc.gpsimd.tensor_mul(sc16[:, c, 0:D_IN], ctrs32[:, c, :], sbc[:])
                t2 = tmp1p.tile([128, D_IN], f32)
                nc.gpsimd.tensor_mul(t2[:], sc16[:, c, 0:D_IN], ctrs32[:, c, :])
                # (tensor_tensor_reduce w/ accum_out wedges the exec unit on
                # HW; tensor_reduce is fine)
                nc.vector.tensor_reduce(
                    negcsq[:, c : c + 1], t2[:],
                    axis=mybir.AxisListType.X, op=Add, negate=True,
                )
            # XBAR out keeps 128 partitions (rows 64..127 junk) so sim and
            # hardware agree on the block mapping; matmuls read rows 0..63.
            lhs1 = constp.tile([128, KC, 128], f16)
            nc.sync.dma_start_transpose(
                lhs1[:], sc16[:].rearrange("p c d -> p (c d)")
            )

            # values staging is only needed once phase 2 of pair 0 starts
            vals_stage = constp.tile([128, KC, D_OUT], f32)
            nc.gpsimd.dma_start(
                vals_stage[:], values[:].rearrange("(c p) v -> p c v", p=128)
            )
            ones_kc = constp.tile([128, KC, 2], f32)
            nc.vector.memset(ones_kc[:], 1.0)
            vals = constp.tile([128, KC, D_OUT + 2], bf16)
            nc.vector.tensor_copy(vals[:, :, 0:D_OUT], vals_stage[:])
            nc.vector.tensor_copy(vals[:, :, D_OUT : D_OUT + 2], ones_kc[:])

            # ---------------- main loop ----------------
            def phase1_chunk(xsT2, E2, c):
                # same chunk of two row-tiles: shared stationary + shared
                # f32 c_sq bias on the single exp
                pe = psA.tile([128, 2, TROWS], f32, tag="psA")
                nc.tensor.matmul(pe[:, 0, :], lhs1[0:D_IN, c, :], xsT2[0][0:D_IN, :, :])
                nc.tensor.matmul(pe[:, 1, :], lhs1[0:D_IN, c, :], xsT2[1][0:D_IN, :, :])
                nc.scalar.activation(
                    E2[:, c, :, :], pe[:], Exp, scale=2.0,
                    bias=negcsq[:, c : c + 1],
                )

            def phase2_subtile(E2, t, gi, ysb, a):
                po = psO.tile([128, D_OUT + 2], f32, tag="psO")
                for c in range(KC):
                    nc.tensor.matmul(
                        po[:],
                        E2[:, c, t, a * 128 : (a + 1) * 128],
                        vals[:, c, :],
                        start=(c == 0),
                        stop=(c == KC - 1),
                    )
                rcp = rcpp.tile([128, 1], f32)
                nc.vector.reciprocal(rcp[:], po[:, D_OUT : D_OUT + 1])
                nc.vector.tensor_scalar_mul(ysb[:, a, :], po[:, 0:D_OUT], rcp[:])
                if a == NSUB - 1:
                    n0 = gi * TROWS
                    nc.gpsimd.dma_start(
                        y[n0 : n0 + TROWS, :].rearrange("(a p) v -> p a v", p=128),
                        ysb[:],
                    )

            Eprev = None
            ysb_pair = [None, None]
            for pi in range(npairs):
                if pi + 1 < npairs:
                    xt_inflight = [
                        issue_x_dma(2 * pi + 2),
                        issue_x_dma(2 * pi + 3),
                    ]
                E2 = Ep.tile([128, KC, 2, TROWS], bf16)
                xsT_next = None
                # 8 phase-1 chunk-pairs interleave 1:1 with the previous
                # pair's 8 phase-2 sub-tiles (engine queues are in-order:
                # the PE streams phase-2 while ACT drains the exp).
                for c in range(KC):
                    phase1_chunk(xsT_cur, E2, c)
                    if c == 2 and pi + 1 < npairs:
                        xsT_next = [
                            assemble_xsT(xt_inflight[0], 2 * pi + 2),
                            assemble_xsT(xt_inflight[1], 2 * pi + 3),
                        ]
                    if Eprev is not None:
                        t, a = divmod(c, NSUB)
                        if a == 0:
                            ysb_pair[t] = yp.tile(
                                [128, NSUB, D_OUT], f32, name="ysb"
                            )
                        phase2_subtile(
                            Eprev, t, 2 * (pi - 1) + t, ysb_pair[t], a
                        )
                Eprev = E2
                if xsT_next is not None:
                    xsT_cur = xsT_next
            for idx in range(2 * NSUB):
                t, a = divmod(idx, NSUB)
                if a == 0:
                    ysb_pair[t] = yp.tile([128, NSUB, D_OUT], f32, name="ysb")
                phase2_subtile(Eprev, t, 2 * (npairs - 1) + t, ysb_pair[t], a)

    nc.compile()
    nc.finalize()
    return nc


def get_nc(use_f32r=USE_F32R, rows=NS, dma="sync", ph2_bf16=True):
    key = ("nc", rows)
    if key not in _cache:
        _cache[key] = _build(rows)
    return _cache[key]


def make_in_maps(x, ctrs, values, s):
    x = np.ascontiguousarray(x, dtype=np.float32)
    ctrs = np.ascontiguousarray(ctrs, dtype=np.float32)
    values = np.ascontiguousarray(values, dtype=np.float32)
    s = np.ascontiguousarray(s, dtype=np.float32)
    return [
        {
            "x": x[i * NS : (i + 1) * NS],
            "ctrs": ctrs,
            "values": values,
            "s": s,
        }
        for i in range(NCORES)
    ]


def run(x, ctrs, values, s, trace=False, use_f32r=USE_F32R, tmpdir=None):
    from concourse.bass_utils import run_bass_kernel_spmd

    nc = get_nc(use_f32r)
    res = run_bass_kernel_spmd(
        nc,
        make_in_maps(x, ctrs, values, s),
        list(range(NCORES)),
        trace=trace,
        tmpdir=tmpdir,
    )
    out = np.concatenate([res.results[i]["y"] for i in range(NCORES)], axis=0)
    return out, res


def kernel(x, ctrs, values, s):
    out, _ = run(x, ctrs, values, s, trace=False)
    return out.astype(np.float32)


# revision 24
# speedup vs baseline: 1.2482x; 1.2482x over previous
"""Trainium2 Bass kernel for the vq_codebook problem.

  dist_sq[n,k] = sum_d (x[n,d]-ctrs[k,d])^2 * s[d]
  out = softmax(-dist_sq, axis=1) @ values

Sharding: data-parallel over N (8192 rows of x per core); ctrs/values/s
replicated on all 8 cores. No collectives (forward only).

Math trick: softmax is shift-invariant, so
  softmax(-dist_sq)[n,k] = softmax(2*cross_s[n,k] - c_sq[k])  with
  cross_s = (x*s) @ ctrs.T,  c_sq[k] = sum_d s[d]*ctrs[k,d]^2.
We compute E = exp(2*(cross_s - 0.5*c_sq)) unnormalized (range-checked:
max exponent ~48 < 88, row-max min ~ -27, so fp32 exp never overflows
and denominators stay normal), then
  y[n,:] = (E.T @ values_aug)[n,:256] / (E.T @ values_aug)[n,256]
with values_aug = [values | ones] so the denominator comes from the same
accumulating matmul.

Layouts: phase 1 runs transposed (k on partitions, n on free) with an
augmented fp16 stationary matrix lhs1 so a single matmul per
128-centroid chunk produces the whole softmax argument; x tiles are
cast to fp16 and transposed on the PE (fp16 transposes stream at 1
cycle/row vs 2 for f32, and fp16 stationaries get fast-weight-load so
the per-matmul LDWEIGHTS hides under the previous stream — measured:
phase-1 cadence 310ns in f32r vs ~226ns in fp16). fp16 operands add
~4e-3 rel err vs the 2e-2 budget (validated numerically).

The c_sq row of the fp16 stationary would quantize the ~55-magnitude
c_sq values to ~0.03 nats of softmax-argument error, so lhs1 carries a
second residual row: row 64 = fp16(-0.5 c_sq) pairs with a ones row of
xsT, row 96 = (exact - fp16)*2^10 pairs with a constant 2^-10 row
(engine writes may only start at partitions 0/32/64/96; rows 65..95 are
zero on both sides). Contraction depth doesn't affect matmul time (cost
is per moving column), so the extra rows are free.

Phase 2 uses bf16 E chunks as the stationary operand against
values_aug, producing y in natural [n, d_out] layout (fp8/DoubleRow was
evaluated numerically and busts the error budget; bf16 is the floor).

Pipelining: phase-1 chunk pairs and the previous tile's phase-2
sub-tiles are interleaved 1:1 in emission order so the PE streams
phase-2 matmuls while the ACT engine drains each pair's exp (~1us).
"""

import os

os.environ.setdefault("JAX_PLATFORMS", "axon")

import numpy as np

N, D_IN, K, D_OUT = 65536, 64, 1024, 256
NCORES = 8
NS = N // NCORES  # 8192 rows per core
TROWS = 512  # rows of x per tile
NTILES = NS // TROWS  # 16
KC = K // 128  # 8 centroid chunks
NSUB = TROWS // 128  # 4 output sub-tiles per tile

USE_F32R = True

_cache = {}


def _build(rows=NS):
    import concourse.bacc as bacc
    import concourse.tile as tile
    from concourse import masks, mybir

    f32 = mybir.dt.float32
    f16 = mybir.dt.float16
    bf16 = mybir.dt.bfloat16
    Exp = mybir.ActivationFunctionType.Exp
    Copy = mybir.ActivationFunctionType.Copy
    Sub = mybir.AluOpType.subtract
    Mult = mybir.AluOpType.mult

    ntiles = rows // TROWS
    nc = bacc.Bacc("TRN2", target_bir_lowering=False, debug=False)
    x = nc.declare_dram_parameter("x", [rows, D_IN], f32, isOutput=False)
    ctrs = nc.declare_dram_parameter("ctrs", [K, D_IN], f32, isOutput=False)
    values = nc.declare_dram_parameter("values", [K, D_OUT], f32, isOutput=False)
    s = nc.declare_dram_parameter("s", [D_IN], f32, isOutput=False)
    y = nc.declare_dram_parameter("y", [rows, D_OUT], f32, isOutput=True)

    with tile.TileContext(nc) as tc:
        with (
            tc.tile_pool(name="const", bufs=1) as constp,
            tc.tile_pool(name="tmp1", bufs=2) as tmp1p,
            tc.tile_pool(name="xt32", bufs=4) as xt32p,
            tc.tile_pool(name="xt16", bufs=3) as xt16p,
            tc.tile_pool(name="E", bufs=3) as Ep,
            tc.tile_pool(name="ysb", bufs=3) as yp,
            tc.tile_pool(name="rcp", bufs=8) as rcpp,
            tc.tile_pool(name="psA", bufs=2, space="PSUM") as psA,
            tc.tile_pool(name="psX", bufs=2, space="PSUM") as psX,
            tc.tile_pool(name="psO", bufs=2, space="PSUM") as psO,
        ):
            # -------- startup-critical prefetch: tile-0 x DMA first --------
            def issue_x_dma(i):
                n0 = i * TROWS
                xt32 = xt32p.tile([128, NSUB, D_IN], f32)
                nc.sync.dma_start(
                    xt32[:], x[n0 : n0 + TROWS, :].rearrange("(a p) d -> p a d", p=128)
                )
                return xt32

            xt32_cur = issue_x_dma(0)

            ident16 = constp.tile([128, 128], f16)
            masks.make_identity(nc, ident16[:])
            ident32 = constp.tile([128, 128], f32)
            masks.make_identity(nc, ident32[:])

            # Persistent triple-buffered phase-1 moving operand. Rows 0..63
            # x^T (written per tile), 64 ones, 96 = 2^-10, 65..95 zero.
            xsT_all = constp.tile([97, 3, TROWS], f16)
            nc.vector.memset(xsT_all[64:96, :, :], 0.0)
            nc.vector.memset(xsT_all[64:65, :, :], 1.0)
            nc.vector.memset(xsT_all[96:97, :, :], 2.0**-10)

            def assemble_xsT(xt32, i):
                xt16 = xt16p.tile([128, NSUB, D_IN], f16)
                nc.vector.tensor_copy(xt16[:], xt32[:])
                slot = i % 3
                for p in range(NSUB // 2):
                    # Paired transpose: [128, 2, 64] -> [128, 128] PSUM with
                    # x_{2p}^T on partitions 0..63 and x_{2p+1}^T on 64..127.
                    xp = psX.tile([128, 128], f16, tag="psX")
                    nc.tensor.transpose(
                        xp[:],
                        xt16[:, 2 * p : 2 * p + 2, :].rearrange("q a d -> q (a d)"),
                        ident16[:],
                    )
                    c0 = 2 * p * 128
                    nc.vector.tensor_copy(
                        xsT_all[0:D_IN, slot, c0 : c0 + 128], xp[0:64, :]
                    )
                    # Upper half shifts partitions 64..127 -> 0..63 via the
                    # engine write crossbar.
                    nc.vector.tensor_copy(
                        xsT_all[0:D_IN, slot, c0 + 128 : c0 + 256], xp[64:128, :]
                    )
                return xsT_all[:, slot, :]

            # ---------------- constants ----------------
            s_col = constp.tile([D_IN, 1], f32)
            nc.sync.dma_start(s_col[:], s[:].rearrange("(p o) -> p o", o=1))
            ctrs_nat = constp.tile([128, KC, D_IN], f32)
            nc.sync.dma_start(
                ctrs_nat[:], ctrs[:].rearrange("(c p) d -> p c d", p=128)
            )

            # lhs1[0:64, c, :] = s[d] * ctrs^T chunk       (d on partitions)
            # lhs1[64, c, :]   = fp16(-0.5 * c_sq) chunk   (k on free)
            # lhs1[96, c, :]   = (-0.5*c_sq - fp16(-0.5*c_sq)) * 2^10
            lhs1 = constp.tile([97, KC, 128], f16)
            nc.vector.memset(lhs1[64:96, :, :], 0.0)
            for c in range(KC):
                tp = psX.tile([D_IN, TROWS], f32, tag="psX")
                nc.tensor.transpose(tp[:, 0:128], ctrs_nat[:, c, :], ident32[:])
                nc.scalar.activation(
                    lhs1[0:D_IN, c, :], tp[:, 0:128], Copy, scale=s_col[:]
                )
                tmp = tmp1p.tile([D_IN, 128], f32)
                nc.scalar.square(tmp[:], tp[:, 0:128])
                csq = psO.tile([1, D_OUT + 2], f32, tag="psO")
                # csq[0, k] = sum_d s[d] * ctrs[k, d]^2  (s_col as stationary)
                nc.tensor.matmul(csq[0:1, 0:128], s_col[:], tmp[:])
                nc.scalar.activation(
                    lhs1[64:65, c, :], csq[0:1, 0:128], Copy, scale=-0.5
                )
                res = tmp1p.tile([1, 128], f32)
                nc.vector.scalar_tensor_tensor(
                    res[:], csq[0:1, 0:128], -0.5, lhs1[64:65, c, :], Mult, Sub
                )
                nc.scalar.activation(lhs1[96:97, c, :], res[:], Copy, scale=1024.0)

            # values staging is only needed once phase 2 of tile 0 starts
            vals_stage = constp.tile([128, KC, D_OUT], f32)
            nc.sync.dma_start(
                vals_stage[:], values[:].rearrange("(c p) v -> p c v", p=128)
            )
            ones_kc = constp.tile([128, KC, 2], f32)
            nc.vector.memset(ones_kc[:], 1.0)
            vals = constp.tile([128, KC, D_OUT + 2], bf16)
            nc.vector.tensor_copy(vals[:, :, 0:D_OUT], vals_stage[:])
            nc.vector.tensor_copy(vals[:, :, D_OUT : D_OUT + 2], ones_kc[:])

            # ---------------- main loop ----------------
            def phase1_pair(xsT, E, c):
                pe = psA.tile([128, 2, TROWS], f32, tag="psA")
                nc.tensor.matmul(pe[:, 0, :], lhs1[:, c, :], xsT)
                nc.tensor.matmul(pe[:, 1, :], lhs1[:, c + 1, :], xsT)
                nc.scalar.activation(E[:, c : c + 2, :], pe[:], Exp, scale=2.0)

            def phase2_subtile(i, E, ysb, a):
                po = psO.tile([128, D_OUT + 2], f32, tag="psO")
                for c in range(KC):
                    nc.tensor.matmul(
                        po[:],
                        E[:, c, a * 128 : (a + 1) * 128],
                        vals[:, c, :],
                        start=(c == 0),
                        stop=(c == KC - 1),
                    )
                rcp = rcpp.tile([128, 1], f32)
                nc.vector.reciprocal(rcp[:], po[:, D_OUT : D_OUT + 1])
                nc.vector.tensor_scalar_mul(ysb[:, a, :], po[:, 0:D_OUT], rcp[:])
                if a == NSUB - 1:
                    n0 = i * TROWS
                    nc.gpsimd.dma_start(
                        y[n0 : n0 + TROWS, :].rearrange("(a p) v -> p a v", p=128),
                        ysb[:],
                    )

            Eprev = None
            ysb_prev = None
            for i in range(ntiles):
                xt32_next = issue_x_dma(i + 1) if i + 1 < ntiles else None
                xsT = assemble_xsT(xt32_cur, i)
                xt32_cur = xt32_next
                Ecur = Ep.tile([128, KC, TROWS], bf16)
                ysb_cur = yp.tile([128, NSUB, D_OUT], f32)
                # Interleave: while ACT drains pair j's exp, the PE streams
                # the previous tile's phase-2 sub-tile j instead of stalling
                # on the next psA buffer.
                for j in range(NSUB):
                    phase1_pair(xsT, Ecur, 2 * j)
                    if Eprev is not None:
                        phase2_subtile(i - 1, Eprev, ysb_prev, j)
                Eprev = Ecur
                ysb_prev = ysb_cur
            for j in range(NSUB):
                phase2_subtile(ntiles - 1, Eprev, ysb_prev, j)

    nc.compile()
    nc.finalize()
    return nc


def get_nc(use_f32r=USE_F32R, rows=NS, dma="sync", ph2_bf16=True):
    key = ("nc", rows)
    if key not in _cache:
        _cache[key] = _build(rows)
    return _cache[key]


def make_in_maps(x, ctrs, values, s):
    x = np.ascontiguousarray(x, dtype=np.float32)
    ctrs = np.ascontiguousarray(ctrs, dtype=np.float32)
    values = np.ascontiguousarray(values, dtype=np.float32)
    s = np.ascontiguousarray(s, dtype=np.float32)
    return [
        {
            "x": x[i * NS : (i + 1) * NS],
            "ctrs": ctrs,
            "values": values,
            "s": s,
        }
        for i in range(NCORES)
    ]


def run(x, ctrs, values, s, trace=False, use_f32r=USE_F32R, tmpdir=None):
    from concourse.bass_utils import run_bass_kernel_spmd

    nc = get_nc(use_f32r)
    res = run_bass_kernel_spmd(
        nc,
        make_in_maps(x, ctrs, values, s),
        list(range(NCORES)),
        trace=trace,
        tmpdir=tmpdir,
    )
    out = np.concatenate([res.results[i]["y"] for i in range(NCORES)], axis=0)
    return out, res


def kernel(x, ctrs, values, s):
    out, _ = run(x, ctrs, values, s, trace=False)
    return out.astype(np.float32)
